# revision 6
# baseline (speedup 1.0000x reference)
"""JuliaSetAttention Trainium2 kernel.

Full attention out = softmax(Q K^T / sqrt(D) + bias[k]) @ V with a
position-dependent bias bias[k] = log(exp(julia_escape_time[k]*scale)+1e-8).

Key identity used: softmax_k(s_k + log u_k) @ V = (sum_k u_k e^{s_k} v_k) /
(sum_k u_k e^{s_k}) with u_k = exp(et_k*scale)+1e-8.  So the bias never has to
be added to the scores: V is pre-scaled by u on the host and a u-column is
appended (for the softmax denominator).  The device kernel is then a pure
exp(QK^T/8) flash-style attention with no max subtraction (scores are
bounded ~|s|<7, exp stays well inside fp32 range).

Sharding: 8 cores = 4 batches x 2 query halves.  Per core:
  q [4096, 64], k [8192, 64], vv [8192, 65] -> out [4096, 64]

Device algorithm (per core), everything fp32:
  - Load K, Q natural; transpose on PE (via identity matmuls) into
    KT2[128, 4096]: pair j holds K^T of k-tile 2j on partitions 0-63 and
    k-tile 2j+1 on partitions 64-127 (for 2-way row-packed matmuls), and
    QT2[128, 4096]: Q^T duplicated on both partition halves.
  - For each q-block of 512 q rows:
      for each k-tile (128 keys): S_T[k, q] = K Q^T via matmul
        (lhsT = K^T slice [64,128], rhs = Q^T slice [64,512]), two k-tiles
        run concurrently on disjoint PE row halves.
      groups of 3 k-tiles (3 PSUM banks) -> one ScalarE exp() op [128,1536]
        PSUM->SBUF (W = exp(S_T/8)).
      accumulate O'^T[65, q] += vv_tile^T W_tile over all 64 k-tiles in PSUM
        (lhsT = vv tile [128,65], rhs = W slice [128,512]).  Row 64 = sum of
        weights l[q].
      epilogue: copy O'^T to SBUF, PE-transpose to [q, 65], divide cols 0-63
        by col 64 (DVE reciprocal + tensor_scalar_mul), DMA out.
"""

import math

import numpy as np

JULIA_ITERS = 64
ESCAPE_RADIUS = 2.0
B, S, D = 4, 8192, 64
NCORES = 8
QSH = S // 2  # query rows per core
NKT = S // 128  # 64 k-tiles
NQB = QSH // 512  # 8 q-blocks per core
GS = 3  # k-tiles per exp() group (= PSUM banks per ACT op)

_CACHE = {}
STATS = {}
PROFILE = False


def _julia_u(c_real, c_imag, scale):
    """exp(bias) = julia_weights + 1e-8; computed with jax on CPU so the
    escape-time iteration matches the reference bit-exactly."""
    import jax
    import jax.numpy as jnp

    cpu = jax.devices("cpu")[0]
    with jax.default_device(cpu):
        x = jnp.linspace(-2.0, 2.0, S)
        zr0 = x
        zi0 = jnp.zeros_like(x)
        escaped0 = jnp.zeros(S, dtype=bool)
        et0 = jnp.ones(S, dtype=x.dtype)
        r2 = ESCAPE_RADIUS * ESCAPE_RADIUS

        def step(carry, it):
            zr, zi, escaped, et = carry
            nzr = zr * zr - zi * zi + c_real
            nzi = 2.0 * zr * zi + c_imag
            zr = jnp.where(escaped, zr, nzr)
            zi = jnp.where(escaped, zi, nzi)
            mag2 = zr * zr + zi * zi
            newly = jnp.logical_and(~escaped, mag2 > r2)
            et = jnp.where(newly, it.astype(et.dtype) / JULIA_ITERS, et)
            escaped = jnp.logical_or(escaped, newly)
            return (zr, zi, escaped, et), None

        (_, _, _, et), _ = jax.lax.scan(
            step, (zr0, zi0, escaped0, et0), jnp.arange(JULIA_ITERS)
        )
        w = jnp.exp(et * jnp.float32(scale))
        u = w + 1e-8
    return np.asarray(u, np.float32)


def _emit(stack, tc, nc, mybir, q_ap, k_ap, vv_ap, out_ap):
    from concourse.masks import make_identity

    dt = mybir.dt.float32
    AF = mybir.ActivationFunctionType

    singles = stack.enter_context(tc.tile_pool(name="singles", bufs=1))
    w_pool = stack.enter_context(tc.tile_pool(name="wpool", bufs=3))
    osb_pool = stack.enter_context(tc.tile_pool(name="osb", bufs=2))
    o_pool = stack.enter_context(tc.tile_pool(name="opool", bufs=4))
    r_pool = stack.enter_context(tc.tile_pool(name="rpool", bufs=4))
    st_pool = stack.enter_context(tc.tile_pool(name="st", bufs=2, space="PSUM"))
    ot_pool = stack.enter_context(tc.tile_pool(name="ot", bufs=2, space="PSUM"))

    ident = singles.tile([128, 128], dt)
    make_identity(nc, ident)

    k_nat = singles.tile([128, NKT, D], dt)
    nc.sync.dma_start(out=k_nat, in_=k_ap.rearrange("(t p) d -> p t d", p=128))
    q_nat = singles.tile([128, QSH // 128, D], dt)
    nc.sync.dma_start(out=q_nat, in_=q_ap.rearrange("(t p) d -> p t d", p=128))
    vv_sb = singles.tile([128, NKT, D + 1], dt)
    nc.sync.dma_start(out=vv_sb, in_=vv_ap.rearrange("(t p) j -> p t j", p=128))

    KT2 = singles.tile([128, S // 2], dt)
    QT2 = singles.tile([128, QSH], dt)

    # --- prologue: on-chip transposes K -> KT2, Q -> QT2 (12 blocks/round) ---
    npairs = NKT // 2  # 32
    for r0 in range(0, npairs, 12):
        n = min(12, npairs - r0)
        stp = st_pool.tile([128, GS * 512], dt, tag="st", name=f"kprolog{r0}")
        for i in range(n):
            j = r0 + i
            nc.tensor.transpose(
                out=stp[:, i * 128 : (i + 1) * 128],
                in_=k_nat[:, 2 * j : 2 * j + 2, :],
                identity=ident,
            )
        nc.vector.tensor_copy(
            out=KT2[:, r0 * 128 : (r0 + n) * 128], in_=stp[:, 0 : n * 128]
        )
    nqt = QSH // 128  # 32
    for r0 in range(0, nqt, 12):
        n = min(12, nqt - r0)
        stp = st_pool.tile([128, GS * 512], dt, tag="st", name=f"qprolog{r0}")
        for i in range(n):
            t = r0 + i
            nc.tensor.transpose(
                out=stp[0:64, i * 128 : (i + 1) * 128],
                in_=q_nat[:, t, :],
                identity=ident,
            )
        nc.vector.tensor_copy(
            out=QT2[0:64, r0 * 128 : (r0 + n) * 128], in_=stp[0:64, 0 : n * 128]
        )
    # duplicate Q^T onto partitions 64-127 (SBUF->SBUF DMA partition shift)
    nc.sync.dma_start(out=QT2[64:128, :], in_=QT2[0:64, :])

    # --- main loop ---
    ngrp_full = NKT // GS  # 21 full groups of 3, remainder 1
    for qb in range(NQB):
        ot = ot_pool.tile([65, 512], dt, tag="ot", name=f"ot{qb}")
        st_of_group = {}
        for kt in range(NKT):
            g, bk = divmod(kt, GS)
            gsz = GS if g < ngrp_full else NKT - ngrp_full * GS
            if bk == 0:
                st_of_group[g] = st_pool.tile(
                    [128, gsz * 512], dt, tag="st", name=f"st{qb}_{g}"
                )
            j, half = divmod(kt, 2)
            stt = st_of_group[g]
            nc.tensor.matmul(
                stt[:, bk * 512 : (bk + 1) * 512],
                lhsT=KT2[64 * half : 64 * half + 64, j * 128 : (j + 1) * 128],
                rhs=QT2[64 * half : 64 * half + 64, qb * 512 : (qb + 1) * 512],
                start=True,
                stop=True,
            )
            if bk == gsz - 1:
                w = w_pool.tile([128, gsz * 512], dt, tag="w", name=f"w{qb}_{g}")
                nc.scalar.activation(out=w, in_=stt[:], func=AF.Exp, scale=0.125)
                for bb in range(gsz):
                    kt2 = g * GS + bb
                    nc.tensor.matmul(
                        ot[:],
                        lhsT=vv_sb[:, kt2, :],
                        rhs=w[:, bb * 512 : (bb + 1) * 512],
                        start=(kt2 == 0),
                        stop=(kt2 == NKT - 1),
                    )
        # epilogue for this q-block
        o_sb = osb_pool.tile([65, 512], dt, tag="osb", name=f"osb{qb}")
        nc.vector.tensor_copy(out=o_sb, in_=ot[:])
        tr = st_pool.tile([128, 512], dt, tag="st", name=f"tr{qb}")
        for qs in range(4):
            nc.tensor.transpose(
                out=tr[:, qs * 128 : qs * 128 + 65],
                in_=o_sb[:, qs * 128 : (qs + 1) * 128],
                identity=ident[0:65, 0:65],
            )
        for qs in range(4):
            rc = r_pool.tile([128, 1], dt, tag="rc", name=f"rc{qb}_{qs}")
            nc.vector.reciprocal(out=rc, in_=tr[:, qs * 128 + 64 : qs * 128 + 65])
            ob = o_pool.tile([128, D], dt, tag="ob", name=f"ob{qb}_{qs}")
            nc.vector.tensor_scalar_mul(ob, tr[:, qs * 128 : qs * 128 + 64], rc)
            nc.sync.dma_start(
                out=out_ap[qb * 512 + qs * 128 : qb * 512 + (qs + 1) * 128, :],
                in_=ob,
            )


def _build():
    if "nc" in _CACHE:
        return _CACHE["nc"]
    from contextlib import ExitStack

    import concourse.mybir as mybir
    import concourse.tile as tile
    from concourse import bacc

    dt = mybir.dt.float32
    nc = bacc.Bacc(
        "TRN2", target_bir_lowering=False, debug=False, enable_asserts=False
    )
    q = nc.dram_tensor("q", [QSH, D], dt, kind="ExternalInput").ap()
    k = nc.dram_tensor("k", [S, D], dt, kind="ExternalInput").ap()
    vv = nc.dram_tensor("vv", [S, D + 1], dt, kind="ExternalInput").ap()
    out = nc.dram_tensor("out", [QSH, D], dt, kind="ExternalOutput").ap()
    with tile.TileContext(nc) as tc:
        with ExitStack() as stack:
            _emit(stack, tc, nc, mybir, q, k, vv, out)
    nc.compile()
    _CACHE["nc"] = nc
    return nc


def make_in_maps(query, key, value, julia_c_real, julia_c_imag, escape_scale):
    query = np.asarray(query, np.float32)
    key = np.asarray(key, np.float32)
    value = np.asarray(value, np.float32)
    u = _julia_u(
        float(np.asarray(julia_c_real)),
        float(np.asarray(julia_c_imag)),
        float(np.asarray(escape_scale)),
    )
    in_maps = []
    for c in range(NCORES):
        b, h = divmod(c, 2)
        vvb = np.concatenate(
            [value[b] * u[:, None], u[:, None]], axis=1, dtype=np.float32
        )
        in_maps.append(
            {
                "q": np.ascontiguousarray(query[b, h * QSH : (h + 1) * QSH]),
                "k": np.ascontiguousarray(key[b]),
                "vv": vvb,
            }
        )
    return in_maps


def kernel(query, key, value, julia_c_real, julia_c_imag, escape_scale):
    from concourse.bass_utils import run_bass_kernel_spmd

    nc = _build()
    in_maps = make_in_maps(
        query, key, value, julia_c_real, julia_c_imag, escape_scale
    )
    try:
        res = run_bass_kernel_spmd(
            nc, in_maps, core_ids=list(range(NCORES)), trace=PROFILE
        )
    except ModuleNotFoundError:
        res = run_bass_kernel_spmd(
            nc, in_maps, core_ids=list(range(NCORES)), trace=False
        )
    STATS["exec_time_ns"] = res.exec_time_ns
    out = np.empty((B, S, D), np.float32)
    for c in range(NCORES):
        b, h = divmod(c, 2)
        out[b, h * QSH : (h + 1) * QSH] = res.results[c]["out"]
    return out
